# revision 21
# baseline (speedup 1.0000x reference)
"""Box-projection (clamp) kernel for Trainium2, pure data parallel over 8 cores.

Problem: y_pred (4M, 6) f32, constr_para (4M, 4) f32 = [l_x, u_x, l_y, u_y].
out[:, 0:3] = clip(y_pred[:, 0:3], l_x, u_x)
out[:, 3:6] = clip(y_pred[:, 3:6], l_y, u_y)

Strategy: shard the batch dim across 8 NeuronCores. Each core gets an
identical-shape shard of S = 128*3907 = 500,096 rows (core 7's shard
overlaps core 6's by 768 rows so the full 4,000,000 rows are covered with
one SPMD program and no padding). Within a core, rows are laid out
contiguously per partition: a tile of 128*T rows is one contiguous DRAM
block DMA'd to an SBUF tile [128, T*6]. The clamp runs in-place on the
Vector engine: two min/max ops per column triple, with the per-row bound
broadcast along the contiguous inner dim via a step-0 AP (1 elem/cycle).
The kernel is DMA-bound (~32 MB of traffic per core), so the three DMA
issue paths are used as parallel streams: y/c loads alternate across the
two HWDGE rings (sync/scalar) and stores ride the gpsimd SWDGE path,
sustaining ~416 GB/s aggregate per core.
"""

import sys

for _p in ("/opt/trn_rl_repo", "/root/.axon_site/_ro/trn_rl_repo"):
    if _p not in sys.path:
        sys.path.append(_p)

import numpy as np

_P = 128          # SBUF partitions
_TPP = 3907       # rows per partition per core
_S = _P * _TPP    # 500,096 rows per core shard
_NCORES = 8
_T_LIST = [1024, 1024, 1024, 835]  # rows/partition per tile (sums to _TPP)

_PROG_CACHE = {}


def _build_program(t_list, bufs=4, split_store=False):
    """Build the SPMD Tile program for one core's shard."""
    import concourse.tile as tile
    from concourse import bacc, mybir

    tpp = sum(t_list)
    s = _P * tpp
    f32 = mybir.dt.float32

    nc = bacc.Bacc("TRN2", target_bir_lowering=False, debug=False,
                   num_devices=_NCORES)
    y_d = nc.dram_tensor("y", (s, 6), f32, kind="ExternalInput").ap()
    c_d = nc.dram_tensor("c", (s, 4), f32, kind="ExternalInput").ap()
    o_d = nc.dram_tensor("o", (s, 6), f32, kind="ExternalOutput").ap()

    with tile.TileContext(nc) as tc:
        with tc.tile_pool(name="ypool", bufs=bufs) as ypool, \
             tc.tile_pool(name="cpool", bufs=bufs) as cpool:
            r0 = 0
            for idx, t in enumerate(t_list):
                rows = _P * t
                yt = ypool.tile([_P, t * 6], f32, tag="yt")
                ct = cpool.tile([_P, t * 4], f32, tag="ct")
                y_src = y_d[r0:r0 + rows, :].rearrange("(p t) d -> p (t d)", p=_P)
                c_src = c_d[r0:r0 + rows, :].rearrange("(p t) d -> p (t d)", p=_P)
                # Each HWDGE ring is descgen-limited (~300 GB/s); balance
                # the two load streams across both rings, alternating per
                # tile. Stores go out on the gpsimd SWDGE path so a
                # compute-blocked store never head-of-line-blocks a load.
                ring_a = nc.sync if idx % 2 == 0 else nc.scalar
                ring_b = nc.scalar if idx % 2 == 0 else nc.sync
                ring_a.dma_start(yt[:], y_src)
                ring_b.dma_start(ct[:], c_src)

                y3 = yt[:].rearrange("p (t d) -> p t d", d=6)
                c3 = ct[:].rearrange("p (t d) -> p t d", d=4)
                o3 = o_d[r0:r0 + rows, :].rearrange("(p t) d -> p t d", p=_P)
                # Optionally compute+store in two row-halves so the first
                # half's store overlaps the second half's compute.
                halves = [(0, t // 2), (t // 2, t - t // 2)] if split_store \
                    else [(0, t)]
                for lo_r, n_r in halves:
                    sl = y3[:, lo_r:lo_r + n_r, :]
                    cb = c3[:, lo_r:lo_r + n_r, :]
                    # Clamp 3 columns per op: bounds broadcast along the
                    # contiguous inner dim (step-0 AP) to avoid the DVE
                    # AP-walker penalty of inner-dim-1 strided ops.
                    bshape = (_P, n_r, 3)
                    xs, ys = sl[:, :, 0:3], sl[:, :, 3:6]
                    nc.vector.tensor_tensor(
                        xs, xs, cb[:, :, 1:2].broadcast_to(bshape),
                        mybir.AluOpType.min)
                    nc.vector.tensor_tensor(
                        xs, xs, cb[:, :, 0:1].broadcast_to(bshape),
                        mybir.AluOpType.max)
                    nc.vector.tensor_tensor(
                        ys, ys, cb[:, :, 3:4].broadcast_to(bshape),
                        mybir.AluOpType.min)
                    nc.vector.tensor_tensor(
                        ys, ys, cb[:, :, 2:3].broadcast_to(bshape),
                        mybir.AluOpType.max)
                    nc.gpsimd.dma_start(o3[:, lo_r:lo_r + n_r, :], sl)
                r0 += rows

    nc.compile()
    return nc


def _get_program():
    key = (tuple(_T_LIST),)
    if key not in _PROG_CACHE:
        _PROG_CACHE[key] = _build_program(_T_LIST, split_store=True)
    return _PROG_CACHE[key]


def kernel(y_pred: np.ndarray, constr_para: np.ndarray) -> np.ndarray:
    from concourse.bass_utils import run_bass_kernel_spmd

    batch = y_pred.shape[0]
    y_pred = np.ascontiguousarray(y_pred, dtype=np.float32)
    constr_para = np.ascontiguousarray(constr_para, dtype=np.float32)

    offs = [min(i * _S, batch - _S) for i in range(_NCORES)]
    in_maps = [
        {"y": y_pred[o:o + _S], "c": constr_para[o:o + _S]} for o in offs
    ]

    nc = _get_program()
    res = run_bass_kernel_spmd(nc, in_maps, core_ids=list(range(_NCORES))).results

    out = np.empty((batch, 6), dtype=np.float32)
    for o, r in zip(offs, res):
        out[o:o + _S] = r["o"]
    return out
